# revision 27
# baseline (speedup 1.0000x reference)
"""Two-layer GAT on 8 Trainium2 NeuronCores (Bass/Tile).

Strategy (dst-sharded graph parallel):
  - Self-loops appended; dst nodes greedy-assigned to 80 bins (8 cores x 10
    supertiles, 125 nodes each) balancing per-bin edge counts -> uniform
    T=ceil(max_bin_edges/128) edge tiles per supertile. Host permutes the
    output back at the end.
  - Phase A (replicated): table1[n] = [x@W1 (1024, h-major) | x@Ws (8 a_s) |
    x@Wd (8 a_d) | ex slot | pad] in f16, one 1152-col row per node.
  - Phase B1 per supertile: ONE dma_gather of source rows; one-hot S / S^T
    built on-device from tiny dst-slot index loads via is_equal; a_d expanded
    edge-wise with an S^T matmul; logits -> Lrelu -> Exp on ACT; per-(edge,
    head) scaling via one broadcast tensor_tensor on DVE; scatter-sum +
    denominators via S matmuls into PSUM; ELU; h2 = elu @ W2 (8 transposes +
    matmuls); tp rows [h2|a_s2] written to tpl.
  - AllGather of tpl in 2 chunks (after supertile 4 and 9) overlapped with B1.
  - Phase B2 mirrors B1 with a single head over the f16 tp table.
"""
import heapq
import sys

sys.path.insert(0, "/opt/trn_rl_repo")

import numpy as np

import concourse.bacc as bacc
import concourse.mybir as mybir
from concourse import tile as tile_mod
from concourse.bass_utils import run_bass_kernel_spmd
from concourse.tile import TileContext
from concourse.vector_clock import ScopedClock

# ---------------------------------------------------------------- constants
N, E, FIN = 10000, 160000, 256
H1, C1, C2 = 8, 128, 64
D1 = H1 * C1                      # 1024
ROW1 = 1152                       # table1 row (f16, 2304B)
COL_AS = 1024
COL_AD = 1032
COL_EX = 1040
ROW2 = 128                        # tp row (f16, 256B)
NEG = 0.2
NCORES = 8
NST = 10
NB = NCORES * NST
CAP = N // NB                     # 125
AGCHUNK = 5
HALF = NCORES * AGCHUNK * 128     # 5120 rows per AG chunk in tpg
MCH = 79
NPAD = MCH * 128

f16, f32 = np.float16, np.float32

# ------------------------------------------------- walrus 1-wait workaround


def _wait_cap(inst) -> int:
    return 2 if isinstance(inst, mybir.InstEventSemaphore) else 1


def _pop_appended(nc, inst):
    for f in nc.m.functions:
        for bb in f.blocks:
            if bb.instructions and bb.instructions[-1] is inst:
                bb.instructions.pop()
                return
    for f in nc.m.functions:
        for bb in f.blocks:
            if inst in bb.instructions:
                bb.instructions.remove(inst)
                return


def legalize_waits(nc):
    """This walrus build accepts one sync wait per instruction (two for
    EventSemaphore); hoist excess waits onto same-engine nops."""
    for f in nc.m.functions:
        for bb in f.blocks:
            new_insts = []
            for inst in list(bb.instructions):
                si = inst.sync_info
                waits = list(si.on_wait) if si is not None and si.on_wait else []
                cap = _wait_cap(inst)
                if len(waits) > cap:
                    si.on_wait = waits[:cap]
                    for w in waits[cap:]:
                        nop = nc.engines[inst.engine].nop()
                        nop.ins.sync_info = mybir.SyncInfo(on_wait=[w], on_update=[])
                        _pop_appended(nc, nop.ins)
                        new_insts.append(nop.ins)
                new_insts.append(inst)
            bb.instructions[:] = new_insts


def _patched_drain_and_barrier(self, tick_clock, wait_clock):
    nc = self.nc
    drain_inst = nc.sync.drain()
    wait_clock.add_sem_waits(
        drain_inst.ins, ScopedClock({None: tick_clock.global_clock})
    )
    si = drain_inst.ins.sync_info
    waits = list(si.on_wait) if si is not None and si.on_wait else []
    if len(waits) > 1:
        si.on_wait = waits[:1]
        bb = nc.cur_bb.bb
        nops = []
        for w in waits[1:]:
            nop = nc.sync.nop()
            nop.ins.sync_info = mybir.SyncInfo(on_wait=[w], on_update=[])
            nops.append(nop.ins)
        insts = bb.instructions
        insts.remove(drain_inst.ins)
        insts.append(drain_inst.ins)

    nc.all_engine_barrier()
    assert self.sems is not None
    popped = nc._tile_sem_poison_stack.pop()
    assert popped is self._sem_poison
    nc.clear_and_free_semaphores(list(self.sems.allocated().values()))
    nc.all_engine_barrier()


tile_mod.TileContext._drain_and_barrier = _patched_drain_and_barrier

# ---------------------------------------------------------------- host prep


def _balance(dst_all):
    deg = np.bincount(dst_all, minlength=N)
    order = np.argsort(-deg, kind="stable")
    heap = [(0, b) for b in range(NB)]
    heapq.heapify(heap)
    assign = np.empty(N, np.int32)
    slot = np.empty(N, np.int32)
    cnt = np.zeros(NB, np.int32)
    for n in order:
        load, b = heapq.heappop(heap)
        assign[n] = b
        slot[n] = cnt[b]
        cnt[b] += 1
        if cnt[b] < CAP:
            heapq.heappush(heap, (load + int(deg[n]), b))
    return assign, slot


def _wrap(idx):
    """[..., M] int16 -> [..., 128, M//16] (16-partition wrap, 8 replicas)."""
    M = idx.shape[-1]
    out = np.zeros(idx.shape[:-1] + (128, M // 16), np.int16)
    i = np.arange(M)
    for rep in range(8):
        out[..., 16 * rep + (i % 16), i // 16] = idx
    return out


def _edge_struct(edge_index):
    src = np.concatenate([np.asarray(edge_index[0]), np.arange(N)]).astype(np.int64)
    dst = np.concatenate([np.asarray(edge_index[1]), np.arange(N)]).astype(np.int64)
    assign, slot = _balance(dst)
    core_of = assign // NST
    st_of = assign % NST
    pos = st_of * 128 + slot
    tpgrow = core_of * (NST * 128) + pos

    b_e = assign[dst]
    order_e = np.argsort(b_e, kind="stable")
    loads = np.bincount(b_e, minlength=NB)
    T = int(np.ceil(loads.max() / 128))
    NE = T * 128

    idx1 = np.zeros((NCORES, NST, NE), np.int16)
    idx2 = np.zeros((NCORES, NST, NE), np.int16)
    dlocP = np.full((NCORES, NST, NE), -1, np.int16)
    bounds = np.concatenate([[0], np.cumsum(loads)])
    es_ = src[order_e]
    ed_ = dst[order_e]
    for b in range(NB):
        k, s = b // NST, b % NST
        lo, hi = bounds[b], bounds[b + 1]
        n = hi - lo
        idx1[k, s, :n] = es_[lo:hi]
        idx2[k, s, :n] = tpgrow[es_[lo:hi]]
        dlocP[k, s, :n] = slot[ed_[lo:hi]]

    dP = np.ascontiguousarray(
        dlocP.reshape(NCORES, NST, T, 128).transpose(0, 1, 3, 2).astype(f16))
    dF = np.ascontiguousarray(np.broadcast_to(
        dlocP.astype(f16)[:, :, None, :], (NCORES, NST, 128, NE)))

    node_at = np.zeros((NCORES, NST * 128), np.int64)
    valid = np.zeros((NCORES, NST * 128), bool)
    node_at[core_of, pos] = np.arange(N)
    valid[core_of, pos] = True
    idxd = np.where(valid, node_at, 0).astype(np.int16)

    return dict(idx1=_wrap(idx1), idx2=_wrap(idx2), idxd=_wrap(idxd),
                dP=dP, dF=dF, T=T, node_at=node_at, valid=valid)


def _host_params(x, W1, att_src1, att_dst1, b1, W2, att_src2, att_dst2, b2):
    x = np.asarray(x, f32)
    xT = np.zeros((FIN, NPAD), f16)
    xT[:, :N] = x.T.astype(f16)

    W1_64 = np.asarray(W1, np.float64)
    Ws = np.stack([W1_64[:, h * C1:(h + 1) * C1]
                   @ np.asarray(att_src1, np.float64)[h] for h in range(H1)], 1)
    Wd = np.stack([W1_64[:, h * C1:(h + 1) * C1]
                   @ np.asarray(att_dst1, np.float64)[h] for h in range(H1)], 1)
    W1i = np.zeros((FIN, ROW1), f16)
    W1i[:, :D1] = np.asarray(W1, f32).astype(f16)
    W1i[:, COL_AS:COL_AS + 8] = Ws.astype(f16)
    W1i[:, COL_AD:COL_AD + 8] = Wd.astype(f16)

    W2_64 = np.asarray(W2, np.float64)
    w2s = W2_64 @ np.asarray(att_src2, np.float64)[0]
    w2d = W2_64 @ np.asarray(att_dst2, np.float64)[0]
    W2e = np.zeros((D1, 66), f16)
    W2e[:, 0:64] = np.asarray(W2, f32).astype(f16)
    W2e[:, 64] = w2s.astype(f16)
    W2e[:, 65] = w2d.astype(f16)

    b1r = np.broadcast_to(np.asarray(b1, f32), (128, D1)).copy()
    b2r = np.broadcast_to(np.asarray(b2, f32), (128, C2)).copy()
    iota = np.broadcast_to(np.arange(128, dtype=f16), (128, 128)).copy()
    pidx = np.arange(128, dtype=f16).reshape(128, 1).copy()
    eye = np.eye(128, dtype=f32)
    return dict(xT=xT, W1i=W1i, W2e=W2e, b1r=b1r, b2r=b2r, iota=iota,
                pidx=pidx, eye=eye)


# ------------------------------------------------------------- bass program
_prog_cache = {}


def _build(T, stage="full"):
    dt = mybir.dt
    Alu = mybir.AluOpType
    Act = mybir.ActivationFunctionType
    NE = T * 128

    nc = bacc.Bacc("TRN2", target_bir_lowering=False, debug=False,
                   num_devices=NCORES)
    xT = nc.dram_tensor("xT", [FIN, NPAD], dt.float16, kind="ExternalInput")
    W1i = nc.dram_tensor("W1i", [FIN, ROW1], dt.float16, kind="ExternalInput")
    W2e = nc.dram_tensor("W2e", [D1, 66], dt.float16, kind="ExternalInput")
    b1r = nc.dram_tensor("b1r", [128, D1], dt.float32, kind="ExternalInput")
    b2r = nc.dram_tensor("b2r", [128, C2], dt.float32, kind="ExternalInput")
    iota = nc.dram_tensor("iota", [128, 128], dt.float16, kind="ExternalInput")
    pidx = nc.dram_tensor("pidx", [128, 1], dt.float16, kind="ExternalInput")
    eye = nc.dram_tensor("eye", [128, 128], dt.float32, kind="ExternalInput")
    idx1 = nc.dram_tensor("idx1", [NST, 128, T * 8], dt.int16, kind="ExternalInput")
    idx2 = nc.dram_tensor("idx2", [NST, 128, T * 8], dt.int16, kind="ExternalInput")
    idxd = nc.dram_tensor("idxd", [128, NST * 8], dt.int16, kind="ExternalInput")
    dPt = nc.dram_tensor("dP", [NST, 128, T], dt.float16, kind="ExternalInput")
    dFt = nc.dram_tensor("dF", [NST, 128, NE], dt.float16, kind="ExternalInput")

    table1 = nc.dram_tensor("table1", [N, ROW1], dt.float16)
    tpl = nc.dram_tensor("tpl", [NST * 128, ROW2], dt.float16)
    tpg = nc.dram_tensor("tpg", [2 * HALF, ROW2], dt.float16, addr_space="Shared")
    out = nc.dram_tensor("out", [NST * 128, C2], dt.float32, kind="ExternalOutput")
    if stage != "full":
        dbgA = nc.dram_tensor("dbgA", [128, ROW1], dt.float16, kind="ExternalOutput")
        dbgT = nc.dram_tensor("dbgT", [NST * 128, 66], dt.float32, kind="ExternalOutput")
        dbgG = nc.dram_tensor("dbgG", [4, 128, ROW2], dt.float16, kind="ExternalOutput")
    if stage == "B2D":
        dbgG2 = nc.dram_tensor("dbgG2", [128, ROW2 * T], dt.float16,
                               kind="ExternalOutput")
        dbgS2 = nc.dram_tensor("dbgS2", [128, 4 * T], dt.float32,
                               kind="ExternalOutput")
        dbgU2 = nc.dram_tensor("dbgU2", [128, C2 + 2], dt.float32,
                               kind="ExternalOutput")

    with TileContext(nc) as tc:
        with tc.tile_pool(name="const", bufs=1) as cp:
            w1i_sb = cp.tile([128, 2, ROW1], dt.float16)
            nc.sync.dma_start(w1i_sb[:], W1i.ap().rearrange("(j p) c -> p j c", p=128))
            w2e_sb = cp.tile([128, 8, 66], dt.float16)
            nc.sync.dma_start(w2e_sb[:], W2e.ap().rearrange("(j p) c -> p j c", p=128))
            b1_sb = cp.tile([128, D1], dt.float32)
            nc.sync.dma_start(b1_sb[:], b1r[:])
            b2_sb = cp.tile([128, C2], dt.float32)
            nc.sync.dma_start(b2_sb[:], b2r[:])
            iota_sb = cp.tile([128, 128], dt.float16)
            nc.sync.dma_start(iota_sb[:], iota[:])
            pidx_sb = cp.tile([128, 1], dt.float16)
            nc.sync.dma_start(pidx_sb[:], pidx[:])
            eye_sb = cp.tile([128, 128], dt.float32)
            nc.sync.dma_start(eye_sb[:], eye[:])
            adloc_sb = cp.tile([128, NST, 8], dt.float16)
            ad2_sb = cp.tile([128, NST], dt.float16)
            # dst-slot index tables for the on-device one-hot builds,
            # prefetched once and shared by B1 and B2
            dPall = cp.tile([128, NST, T], dt.float16)
            nc.sync.dma_start(dPall[:], dPt.ap().rearrange("s p t -> p s t"))
            dFall = cp.tile([128, NST, NE], dt.float16)
            nc.sync.dma_start(dFall[:], dFt.ap().rearrange("s p e -> p s e"))

            # ---------------- phase A: h1/score table ----------------
            with (
                tc.tile_pool(name="xa", bufs=3) as xap,
                tc.tile_pool(name="ha", bufs=3) as hap,
                tc.tile_pool(name="pa", bufs=2, space="PSUM") as pap,
            ):
                for ii in range((MCH + 1) // 2):
                    i0 = ii * 2
                    nch = min(2, MCH - i0)
                    xf = xap.tile([128, 2, 256], dt.float16, tag="xf")
                    nc.sync.dma_start(
                        xf[:, :, 0:nch * 128],
                        xT.ap()[:, i0 * 128:(i0 + nch) * 128]
                        .rearrange("(j p) c -> p j c", p=128),
                    )
                    for c in range(nch):
                        i = i0 + c
                        rows = min(128, N - i * 128)
                        ph = pap.tile([128, COL_EX], dt.float32)
                        for j in range(2):
                            for s0, s1 in ((0, 512), (512, 1024),
                                           (1024, COL_EX)):
                                nc.tensor.matmul(
                                    ph[:, s0:s1],
                                    xf[:, j, c * 128:(c + 1) * 128],
                                    w1i_sb[:, j, s0:s1],
                                    start=(j == 0), stop=(j == 1))
                        h1s = hap.tile([128, ROW1], dt.float16, tag="h1s")
                        nc.vector.tensor_copy(h1s[:, 0:520], ph[:, 0:520])
                        nc.scalar.activation(h1s[:, 520:COL_EX],
                                             ph[:, 520:COL_EX], Act.Copy)
                        nc.scalar.dma_start(
                            table1.ap()[i * 128:i * 128 + rows, 0:COL_EX],
                            h1s[0:rows, 0:COL_EX]
                        )
                if stage != "full":
                    da = xap.tile([128, ROW1], dt.float16, tag="da")
                    nc.sync.dma_start(da[:], table1.ap()[0:128, :])
                    nc.sync.dma_start(dbgA[:], da[:])

            # dst-local a_d for this core's 1280 slots
            with tc.tile_pool(name="adg", bufs=1) as adgp:
                idd = adgp.tile([128, NST * 8], dt.int16)
                nc.sync.dma_start(idd[:], idxd[:])
                adg = adgp.tile([128, NST, ROW1], dt.float16)
                nc.gpsimd.dma_gather(adg[:], table1.ap(), idd[:],
                                     NST * 128, NST * 128, ROW1,
                                     single_packet=False)
                nc.vector.tensor_copy(adloc_sb[:],
                                      adg[:, :, COL_AD:COL_AD + 8])

            # ---------------- phase B1: layer-1 edge pass ----------------
            run_b1 = stage != "A"
            with (
                tc.tile_pool(name="ixp", bufs=2) as ixp,
                tc.tile_pool(name="ssp", bufs=2) as ssp,
                tc.tile_pool(name="stp", bufs=2) as stp,
                tc.tile_pool(name="gp", bufs=2) as gp,
                tc.tile_pool(name="scp", bufs=2) as scp,
                tc.tile_pool(name="adp", bufs=2, space="PSUM") as adpp,
                tc.tile_pool(name="up", bufs=1, space="PSUM") as upp,
                tc.tile_pool(name="o1p", bufs=1) as o1p,
                tc.tile_pool(name="etp", bufs=1) as etp,
                tc.tile_pool(name="tpp", bufs=2, space="PSUM") as tpp,
                tc.tile_pool(name="h2pp", bufs=1, space="PSUM") as h2pp,
                tc.tile_pool(name="tps", bufs=2) as tpsp,
            ):
                def b1_front(s):
                    """gather + one-hot build + logits + scale for supertile s"""
                    ix = ixp.tile([128, T * 8], dt.int16, tag="ix")
                    nc.sync.dma_start(ix[:], idx1.ap()[s])
                    S_sb = ssp.tile([128, T, 128], dt.float16, tag="S")
                    nc.vector.tensor_tensor(
                        S_sb[:],
                        iota_sb[:, None, :].broadcast_to([128, T, 128]),
                        dPall[:, s, :, None].broadcast_to([128, T, 128]),
                        Alu.is_equal)
                    ST_sb = stp.tile([128, NE], dt.float16, tag="ST")
                    nc.vector.tensor_tensor(
                        ST_sb[:], dFall[:, s, :],
                        pidx_sb[:].broadcast_to([128, NE]),
                        Alu.is_equal)
                    g = gp.tile([128, T, ROW1], dt.float16, tag="g")
                    nc.gpsimd.dma_gather(g[:], table1.ap(), ix[:], NE, NE,
                                         ROW1, single_packet=False)
                    # a_d expand: adps[e, (t h)] = S^T-matmul
                    adps = adpp.tile([128, T * 8], dt.float32, tag="adps")
                    STv = ST_sb[:].rearrange("p (t e) -> p t e", t=T)
                    for t in range(T):
                        nc.tensor.matmul(adps[:, t * 8:(t + 1) * 8],
                                         STv[:, t, :], adloc_sb[:, s, :],
                                         start=True, stop=True)
                    # logits -> exp
                    sc = scp.tile([128, T * 8], dt.float32, tag="sc")
                    nc.vector.tensor_copy(sc[:], g[:, :, COL_AS:COL_AS + 8])
                    nc.vector.tensor_tensor(sc[:], sc[:], adps[:], Alu.add)
                    lr = scp.tile([128, T * 8], dt.float32, tag="lr")
                    nc.vector.tensor_scalar_mul(lr[:], sc[:], NEG)
                    nc.vector.tensor_max(lr[:], lr[:], sc[:])
                    ex = scp.tile([128, T * 8], dt.float16, tag="ex")
                    nc.scalar.activation(ex[:], lr[:], Act.Exp)
                    # scale features by exp; stash exp in the ex slot
                    gv = g[:].rearrange("p t (h c) -> p t h c", c=C1)
                    exv = ex[:].rearrange("p (t h) -> p t h", h=8)
                    nc.vector.tensor_tensor(
                        gv[:, :, 0:8, :], gv[:, :, 0:8, :],
                        exv[:, :, :, None].broadcast_to([128, T, 8, C1]),
                        Alu.mult)
                    nc.vector.tensor_copy(g[:, :, COL_EX:COL_EX + 8], exv[:])
                    return s, S_sb, g

                def b1_back(st):
                    """scatter + softmax-normalize + ELU + h2 + tp write"""
                    s, S_sb, g = st
                    u = upp.tile([128, COL_EX + 8], dt.float32, tag="u")
                    for t in range(T):
                        for s0, s1 in ((0, 512), (512, 1024),
                                       (1024, COL_EX + 8)):
                            nc.tensor.matmul(u[:, s0:s1], S_sb[:, t, :],
                                             g[:, t, s0:s1],
                                             start=(t == 0), stop=(t == T - 1))
                    rc = scp.tile([128, 8], dt.float32, tag="rc")
                    nc.vector.tensor_scalar(
                        out=rc[:], in0=u[:, COL_EX:COL_EX + 8],
                        scalar1=1e-12, scalar2=None, op0=Alu.add)
                    nc.vector.reciprocal(rc[:], rc[:])
                    o1 = o1p.tile([128, D1], dt.float32, tag="o1")
                    o1v = o1[:].rearrange("p (h c) -> p h c", c=C1)
                    uv = u[:, 0:D1].rearrange("p (h c) -> p h c", c=C1)
                    nc.vector.tensor_tensor(
                        o1v[:], uv[:],
                        rc[:, :, None].broadcast_to([128, 8, C1]), Alu.mult)
                    nc.vector.tensor_add(o1[:], o1[:], b1_sb[:])
                    # ELU
                    r = o1p.tile([128, D1], dt.float32, tag="relu")
                    nc.scalar.activation(r[:], o1[:], Act.Relu)
                    nc.vector.tensor_sub(o1[:], o1[:], r[:])
                    ee = o1p.tile([128, D1], dt.float32, tag="ee")
                    nc.scalar.activation(ee[:], o1[:], Act.Exp)
                    elu = o1p.tile([128, D1], dt.float32, tag="elu")
                    nc.vector.scalar_tensor_tensor(elu[:], ee[:], -1.0, r[:],
                                                   Alu.add, Alu.add)
                    # h2 = elu @ W2e via 8 transposes
                    eluT = etp.tile([128, 8, 128], dt.float16, tag="eluT")
                    for j in range(8):
                        tp_ps = tpp.tile([128, 128], dt.float32, tag="tp")
                        nc.tensor.transpose(tp_ps[:],
                                            elu[:, j * 128:(j + 1) * 128],
                                            eye_sb[:])
                        nc.scalar.activation(eluT[:, j, :], tp_ps[:], Act.Copy)
                    h2p = h2pp.tile([128, 66], dt.float32, tag="h2p")
                    for j in range(8):
                        nc.tensor.matmul(h2p[:], eluT[:, j, :], w2e_sb[:, j, :],
                                         start=(j == 0), stop=(j == 7))
                    tp_sb = tpsp.tile([128, 65], dt.float16, tag="tpsb")
                    nc.vector.tensor_copy(tp_sb[:], h2p[:, 0:65])
                    nc.vector.tensor_copy(ad2_sb[:, s:s + 1], h2p[:, 65:66])
                    nc.scalar.dma_start(
                        tpl.ap()[s * 128:(s + 1) * 128, 0:65], tp_sb[:])
                    if stage != "full":
                        tp_dbg = tpsp.tile([128, 66], dt.float32, tag="tpdbg")
                        nc.vector.tensor_copy(tp_dbg[:], h2p[:])
                        nc.sync.dma_start(
                            dbgT.ap()[s * 128:(s + 1) * 128, :], tp_dbg[:])

                prev = None
                for s in range(NST if run_b1 else 0):
                    st = b1_front(s)
                    if prev is not None:
                        b1_back(prev)
                    prev = st
                if prev is not None:
                    b1_back(prev)

                if run_b1 and stage in ("full", "AG", "B2D"):
                    nc.gpsimd.collective_compute(
                        "AllGather", Alu.bypass,
                        ins=[tpl[:]],
                        outs=[tpg[:]],
                        replica_groups=[list(range(NCORES))],
                    )

            # ---------------- phase B2: layer-2 edge pass ----------------
            with (
                tc.tile_pool(name="ixp2", bufs=2) as ixp2,
                tc.tile_pool(name="ssp2", bufs=2) as ssp2,
                tc.tile_pool(name="stp2", bufs=2) as stp2,
                tc.tile_pool(name="g2p", bufs=2) as g2p,
                tc.tile_pool(name="sc2p", bufs=2) as sc2p,
                tc.tile_pool(name="adp2", bufs=2, space="PSUM") as adpp2,
                tc.tile_pool(name="u2p", bufs=2, space="PSUM") as u2pp,
                tc.tile_pool(name="o2p", bufs=2) as o2p,
            ):
                def b2_front(s):
                    ix2 = ixp2.tile([128, T * 8], dt.int16, tag="ix2")
                    nc.sync.dma_start(ix2[:], idx2.ap()[s])
                    S_sb = ssp2.tile([128, T, 128], dt.float16, tag="S2")
                    nc.vector.tensor_tensor(
                        S_sb[:],
                        iota_sb[:, None, :].broadcast_to([128, T, 128]),
                        dPall[:, s, :, None].broadcast_to([128, T, 128]),
                        Alu.is_equal)
                    ST_sb = stp2.tile([128, NE], dt.float16, tag="ST2")
                    nc.vector.tensor_tensor(
                        ST_sb[:], dFall[:, s, :],
                        pidx_sb[:].broadcast_to([128, NE]),
                        Alu.is_equal)
                    g2 = g2p.tile([128, T, ROW2], dt.float16, tag="g2")
                    nc.gpsimd.dma_gather(g2[:], tpg.ap(), ix2[:], NE, NE,
                                         ROW2, single_packet=False)
                    ad2e = adpp2.tile([128, T], dt.float32, tag="ad2e")
                    STv = ST_sb[:].rearrange("p (t e) -> p t e", t=T)
                    for t in range(T):
                        nc.tensor.matmul(ad2e[:, t:t + 1], STv[:, t, :],
                                         ad2_sb[:, s:s + 1],
                                         start=True, stop=True)
                    sc2 = sc2p.tile([128, T], dt.float32, tag="sc2")
                    nc.vector.tensor_copy(sc2[:], g2[:, :, 64])
                    nc.vector.tensor_tensor(sc2[:], sc2[:], ad2e[:], Alu.add)
                    l2 = sc2p.tile([128, T], dt.float32, tag="l2")
                    nc.vector.tensor_scalar_mul(l2[:], sc2[:], NEG)
                    nc.vector.tensor_max(l2[:], l2[:], sc2[:])
                    e2 = sc2p.tile([128, T], dt.float16, tag="e2")
                    nc.scalar.activation(e2[:], l2[:], Act.Exp)
                    nc.vector.tensor_tensor(
                        g2[:, :, 0:C2], g2[:, :, 0:C2],
                        e2[:, :, None].broadcast_to([128, T, C2]), Alu.mult)
                    nc.vector.tensor_copy(g2[:, :, C2 + 1], e2[:])
                    return s, S_sb, g2, ad2e, sc2, l2, e2

                def b2_back(st):
                    s, S_sb, g2, ad2e, sc2, l2, e2 = st
                    # one accumulation group per t (PSUM groups are
                    # bank-granular): cols 0:64 feats, 64 junk, 65 denom
                    u2 = u2pp.tile([128, C2 + 2], dt.float32, tag="u2")
                    for t in range(T):
                        nc.tensor.matmul(u2[:], S_sb[:, t, :],
                                         g2[:, t, 0:C2 + 2],
                                         start=(t == 0), stop=(t == T - 1))
                    if stage == "B2D" and s == 0:
                        nc.sync.dma_start(
                            dbgG2[:], g2[:].rearrange("p t c -> p (t c)"))
                        sdump = sc2p.tile([128, 4 * T], dt.float32, tag="sd")
                        nc.vector.tensor_copy(sdump[:, 0:T], ad2e[:])
                        nc.vector.tensor_copy(sdump[:, T:2 * T], sc2[:])
                        nc.vector.tensor_copy(sdump[:, 2 * T:3 * T], l2[:])
                        nc.vector.tensor_copy(sdump[:, 3 * T:4 * T], e2[:])
                        nc.sync.dma_start(dbgS2[:], sdump[:])
                        u2dump = o2p.tile([128, C2 + 2], dt.float32, tag="ud")
                        nc.vector.tensor_copy(u2dump[:], u2[:])
                        nc.sync.dma_start(dbgU2[:], u2dump[:])
                    rc2 = sc2p.tile([128, 1], dt.float32, tag="rc2")
                    nc.vector.tensor_scalar(
                        out=rc2[:], in0=u2[:, C2 + 1:C2 + 2],
                        scalar1=1e-12, scalar2=None, op0=Alu.add)
                    nc.vector.reciprocal(rc2[:], rc2[:])
                    o2 = o2p.tile([128, C2], dt.float32, tag="o2")
                    nc.vector.tensor_scalar_mul(o2[:], u2[:, 0:C2],
                                                rc2[:, 0:1])
                    nc.vector.tensor_add(o2[:], o2[:], b2_sb[:])
                    nc.sync.dma_start(
                        out.ap()[s * 128:(s + 1) * 128, :], o2[:])

                prev2 = None
                for s in range(NST if stage in ("full", "B2D") else 0):
                    st2 = b2_front(s)
                    if prev2 is not None:
                        b2_back(prev2)
                    prev2 = st2
                if prev2 is not None:
                    b2_back(prev2)

    nc.compile()
    legalize_waits(nc)
    return nc


def _get_prog(T):
    import os
    stage = os.environ.get("KERNEL_STAGE", "full")
    key = (T, stage)
    if key not in _prog_cache:
        _prog_cache[key] = _build(T, stage)
    return _prog_cache[key]


# ------------------------------------------------------------------ kernel
def kernel(x, edge_index, W1, att_src1, att_dst1, b1, W2, att_src2, att_dst2,
           b2, _run_kwargs=None):
    edge_index = np.asarray(edge_index)
    es = _edge_struct(edge_index)
    params = _host_params(x, W1, att_src1, att_dst1, b1, W2, att_src2,
                          att_dst2, b2)
    T = es["T"]
    nc = _get_prog(T)

    in_maps = []
    for k in range(NCORES):
        m = dict(params)
        m["idx1"] = es["idx1"][k]
        m["idx2"] = es["idx2"][k]
        m["idxd"] = es["idxd"][k]
        m["dP"] = es["dP"][k]
        m["dF"] = es["dF"][k]
        in_maps.append(m)

    res = run_bass_kernel_spmd(nc, in_maps, list(range(NCORES)),
                               **(_run_kwargs or {}))
    full = np.zeros((N, C2), f32)
    for k in range(NCORES):
        ok = es["valid"][k]
        full[es["node_at"][k][ok]] = res.results[k]["out"][ok]
    kernel.last_results = res
    return full


# revision 29
# speedup vs baseline: 1.0667x; 1.0667x over previous
"""Two-layer GAT on 8 Trainium2 NeuronCores (Bass/Tile).

Strategy (dst-sharded graph parallel):
  - Self-loops appended; dst nodes greedy-assigned to 80 bins (8 cores x 10
    supertiles, 125 nodes each) balancing per-bin edge counts -> uniform
    T=ceil(max_bin_edges/128) edge tiles per supertile. Host permutes the
    output back at the end.
  - Phase A (replicated): table1[n] = [x@W1 (1024, h-major) | x@Ws (8 a_s) |
    x@Wd (8 a_d) | ex slot | pad] in f16, one 1152-col row per node.
  - Phase B1 per supertile: ONE dma_gather of source rows; one-hot S / S^T
    built on-device from tiny dst-slot index loads via is_equal; a_d expanded
    edge-wise with an S^T matmul; logits -> Lrelu -> Exp on ACT; per-(edge,
    head) scaling via one broadcast tensor_tensor on DVE; scatter-sum +
    denominators via S matmuls into PSUM; ELU; h2 = elu @ W2 (8 transposes +
    matmuls); tp rows [h2|a_s2] written to tpl.
  - AllGather of tpl in 2 chunks (after supertile 4 and 9) overlapped with B1.
  - Phase B2 mirrors B1 with a single head over the f16 tp table.
"""
import heapq
import sys

sys.path.insert(0, "/opt/trn_rl_repo")

import numpy as np

import concourse.bacc as bacc
import concourse.mybir as mybir
from concourse import tile as tile_mod
from concourse.bass_utils import run_bass_kernel_spmd
from concourse.tile import TileContext
from concourse.vector_clock import ScopedClock

# ---------------------------------------------------------------- constants
N, E, FIN = 10000, 160000, 256
H1, C1, C2 = 8, 128, 64
D1 = H1 * C1                      # 1024
ROW1 = 1152                       # table1 row (f16, 2304B)
COL_AS = 1024
COL_AD = 1032
COL_EX = 1040
ROW2 = 128                        # tp row (f16, 256B)
NEG = 0.2
NCORES = 8
NST = 10
NB = NCORES * NST
CAP = N // NB                     # 125
AGCHUNK = 5
HALF = NCORES * AGCHUNK * 128     # 5120 rows per AG chunk in tpg
MCH = 79
NPAD = MCH * 128

f16, f32 = np.float16, np.float32

# ------------------------------------------------- walrus 1-wait workaround


def _wait_cap(inst) -> int:
    return 2 if isinstance(inst, mybir.InstEventSemaphore) else 1


def _pop_appended(nc, inst):
    for f in nc.m.functions:
        for bb in f.blocks:
            if bb.instructions and bb.instructions[-1] is inst:
                bb.instructions.pop()
                return
    for f in nc.m.functions:
        for bb in f.blocks:
            if inst in bb.instructions:
                bb.instructions.remove(inst)
                return


def legalize_waits(nc):
    """This walrus build accepts one sync wait per instruction (two for
    EventSemaphore); hoist excess waits onto same-engine nops."""
    for f in nc.m.functions:
        for bb in f.blocks:
            new_insts = []
            for inst in list(bb.instructions):
                si = inst.sync_info
                waits = list(si.on_wait) if si is not None and si.on_wait else []
                cap = _wait_cap(inst)
                if len(waits) > cap:
                    si.on_wait = waits[:cap]
                    for w in waits[cap:]:
                        nop = nc.engines[inst.engine].nop()
                        nop.ins.sync_info = mybir.SyncInfo(on_wait=[w], on_update=[])
                        _pop_appended(nc, nop.ins)
                        new_insts.append(nop.ins)
                new_insts.append(inst)
            bb.instructions[:] = new_insts


def _patched_drain_and_barrier(self, tick_clock, wait_clock):
    nc = self.nc
    drain_inst = nc.sync.drain()
    wait_clock.add_sem_waits(
        drain_inst.ins, ScopedClock({None: tick_clock.global_clock})
    )
    si = drain_inst.ins.sync_info
    waits = list(si.on_wait) if si is not None and si.on_wait else []
    if len(waits) > 1:
        si.on_wait = waits[:1]
        bb = nc.cur_bb.bb
        nops = []
        for w in waits[1:]:
            nop = nc.sync.nop()
            nop.ins.sync_info = mybir.SyncInfo(on_wait=[w], on_update=[])
            nops.append(nop.ins)
        insts = bb.instructions
        insts.remove(drain_inst.ins)
        insts.append(drain_inst.ins)

    nc.all_engine_barrier()
    assert self.sems is not None
    popped = nc._tile_sem_poison_stack.pop()
    assert popped is self._sem_poison
    nc.clear_and_free_semaphores(list(self.sems.allocated().values()))
    nc.all_engine_barrier()


tile_mod.TileContext._drain_and_barrier = _patched_drain_and_barrier

# ---------------------------------------------------------------- host prep


def _balance(dst_all):
    deg = np.bincount(dst_all, minlength=N)
    order = np.argsort(-deg, kind="stable")
    heap = [(0, b) for b in range(NB)]
    heapq.heapify(heap)
    assign = np.empty(N, np.int32)
    slot = np.empty(N, np.int32)
    cnt = np.zeros(NB, np.int32)
    for n in order:
        load, b = heapq.heappop(heap)
        assign[n] = b
        slot[n] = cnt[b]
        cnt[b] += 1
        if cnt[b] < CAP:
            heapq.heappush(heap, (load + int(deg[n]), b))
    return assign, slot


def _wrap(idx):
    """[..., M] int16 -> [..., 128, M//16] (16-partition wrap, 8 replicas)."""
    M = idx.shape[-1]
    out = np.zeros(idx.shape[:-1] + (128, M // 16), np.int16)
    i = np.arange(M)
    for rep in range(8):
        out[..., 16 * rep + (i % 16), i // 16] = idx
    return out


def _edge_struct(edge_index):
    src = np.concatenate([np.asarray(edge_index[0]), np.arange(N)]).astype(np.int64)
    dst = np.concatenate([np.asarray(edge_index[1]), np.arange(N)]).astype(np.int64)
    assign, slot = _balance(dst)
    core_of = assign // NST
    st_of = assign % NST
    pos = st_of * 128 + slot
    tpgrow = core_of * (NST * 128) + pos

    b_e = assign[dst]
    order_e = np.argsort(b_e, kind="stable")
    loads = np.bincount(b_e, minlength=NB)
    T = int(np.ceil(loads.max() / 128))
    NE = T * 128

    idx1 = np.zeros((NCORES, NST, NE), np.int16)
    idx2 = np.zeros((NCORES, NST, NE), np.int16)
    dlocP = np.full((NCORES, NST, NE), -1, np.int16)
    bounds = np.concatenate([[0], np.cumsum(loads)])
    es_ = src[order_e]
    ed_ = dst[order_e]
    for b in range(NB):
        k, s = b // NST, b % NST
        lo, hi = bounds[b], bounds[b + 1]
        n = hi - lo
        idx1[k, s, :n] = es_[lo:hi]
        idx2[k, s, :n] = tpgrow[es_[lo:hi]]
        dlocP[k, s, :n] = slot[ed_[lo:hi]]

    dP = np.ascontiguousarray(
        dlocP.reshape(NCORES, NST, T, 128).transpose(0, 1, 3, 2).astype(f16))
    dF = np.ascontiguousarray(np.broadcast_to(
        dlocP.astype(f16)[:, :, None, :], (NCORES, NST, 128, NE)))

    node_at = np.zeros((NCORES, NST * 128), np.int64)
    valid = np.zeros((NCORES, NST * 128), bool)
    node_at[core_of, pos] = np.arange(N)
    valid[core_of, pos] = True
    idxd = np.where(valid, node_at, 0).astype(np.int16)

    return dict(idx1=_wrap(idx1), idx2=_wrap(idx2), idxd=_wrap(idxd),
                dP=dP, dF=dF, T=T, node_at=node_at, valid=valid)


def _host_params(x, W1, att_src1, att_dst1, b1, W2, att_src2, att_dst2, b2):
    x = np.asarray(x, f32)
    xT = np.zeros((FIN, NPAD), f16)
    xT[:, :N] = x.T.astype(f16)

    W1_64 = np.asarray(W1, np.float64)
    Ws = np.stack([W1_64[:, h * C1:(h + 1) * C1]
                   @ np.asarray(att_src1, np.float64)[h] for h in range(H1)], 1)
    Wd = np.stack([W1_64[:, h * C1:(h + 1) * C1]
                   @ np.asarray(att_dst1, np.float64)[h] for h in range(H1)], 1)
    W1i = np.zeros((FIN, ROW1), f16)
    W1i[:, :D1] = np.asarray(W1, f32).astype(f16)
    W1i[:, COL_AS:COL_AS + 8] = Ws.astype(f16)
    W1i[:, COL_AD:COL_AD + 8] = Wd.astype(f16)

    W2_64 = np.asarray(W2, np.float64)
    w2s = W2_64 @ np.asarray(att_src2, np.float64)[0]
    w2d = W2_64 @ np.asarray(att_dst2, np.float64)[0]
    W2e = np.zeros((D1, 66), f16)
    W2e[:, 0:64] = np.asarray(W2, f32).astype(f16)
    W2e[:, 64] = w2s.astype(f16)
    W2e[:, 65] = w2d.astype(f16)

    b1r = np.broadcast_to(np.asarray(b1, f32), (128, D1)).copy()
    b2r = np.broadcast_to(np.asarray(b2, f32), (128, C2)).copy()
    iota = np.broadcast_to(np.arange(128, dtype=f16), (128, 128)).copy()
    pidx = np.arange(128, dtype=f16).reshape(128, 1).copy()
    eye = np.eye(128, dtype=f32)
    return dict(xT=xT, W1i=W1i, W2e=W2e, b1r=b1r, b2r=b2r, iota=iota,
                pidx=pidx, eye=eye)


# ------------------------------------------------------------- bass program
_prog_cache = {}


def _build(T, stage="full"):
    dt = mybir.dt
    Alu = mybir.AluOpType
    Act = mybir.ActivationFunctionType
    NE = T * 128

    nc = bacc.Bacc("TRN2", target_bir_lowering=False, debug=False,
                   num_devices=NCORES)
    xT = nc.dram_tensor("xT", [FIN, NPAD], dt.float16, kind="ExternalInput")
    W1i = nc.dram_tensor("W1i", [FIN, ROW1], dt.float16, kind="ExternalInput")
    W2e = nc.dram_tensor("W2e", [D1, 66], dt.float16, kind="ExternalInput")
    b1r = nc.dram_tensor("b1r", [128, D1], dt.float32, kind="ExternalInput")
    b2r = nc.dram_tensor("b2r", [128, C2], dt.float32, kind="ExternalInput")
    iota = nc.dram_tensor("iota", [128, 128], dt.float16, kind="ExternalInput")
    pidx = nc.dram_tensor("pidx", [128, 1], dt.float16, kind="ExternalInput")
    eye = nc.dram_tensor("eye", [128, 128], dt.float32, kind="ExternalInput")
    idx1 = nc.dram_tensor("idx1", [NST, 128, T * 8], dt.int16, kind="ExternalInput")
    idx2 = nc.dram_tensor("idx2", [NST, 128, T * 8], dt.int16, kind="ExternalInput")
    idxd = nc.dram_tensor("idxd", [128, NST * 8], dt.int16, kind="ExternalInput")
    dPt = nc.dram_tensor("dP", [NST, 128, T], dt.float16, kind="ExternalInput")
    dFt = nc.dram_tensor("dF", [NST, 128, NE], dt.float16, kind="ExternalInput")

    table1 = nc.dram_tensor("table1", [N, ROW1], dt.float16)
    tpl = nc.dram_tensor("tpl", [NST * 128, ROW2], dt.float16)
    tpg = nc.dram_tensor("tpg", [2 * HALF, ROW2], dt.float16, addr_space="Shared")
    out = nc.dram_tensor("out", [NST * 128, C2], dt.float32, kind="ExternalOutput")
    if stage != "full":
        dbgA = nc.dram_tensor("dbgA", [128, ROW1], dt.float16, kind="ExternalOutput")
        dbgT = nc.dram_tensor("dbgT", [NST * 128, 66], dt.float32, kind="ExternalOutput")
        dbgG = nc.dram_tensor("dbgG", [4, 128, ROW2], dt.float16, kind="ExternalOutput")
    if stage == "B2D":
        dbgG2 = nc.dram_tensor("dbgG2", [128, ROW2 * T], dt.float16,
                               kind="ExternalOutput")
        dbgS2 = nc.dram_tensor("dbgS2", [128, 4 * T], dt.float32,
                               kind="ExternalOutput")
        dbgU2 = nc.dram_tensor("dbgU2", [128, C2 + 2], dt.float32,
                               kind="ExternalOutput")

    with TileContext(nc) as tc:
        with tc.tile_pool(name="const", bufs=1) as cp:
            w1i_sb = cp.tile([128, 2, ROW1], dt.float16)
            nc.sync.dma_start(w1i_sb[:], W1i.ap().rearrange("(j p) c -> p j c", p=128))
            w2e_sb = cp.tile([128, 8, 66], dt.float16)
            nc.sync.dma_start(w2e_sb[:], W2e.ap().rearrange("(j p) c -> p j c", p=128))
            b1_sb = cp.tile([128, D1], dt.float32)
            nc.sync.dma_start(b1_sb[:], b1r[:])
            b2_sb = cp.tile([128, C2], dt.float32)
            nc.sync.dma_start(b2_sb[:], b2r[:])
            iota_sb = cp.tile([128, 128], dt.float16)
            nc.sync.dma_start(iota_sb[:], iota[:])
            pidx_sb = cp.tile([128, 1], dt.float16)
            nc.sync.dma_start(pidx_sb[:], pidx[:])
            eye_sb = cp.tile([128, 128], dt.float32)
            nc.sync.dma_start(eye_sb[:], eye[:])
            adloc_sb = cp.tile([128, NST, 8], dt.float16)
            ad2_sb = cp.tile([128, NST], dt.float16)
            # dst-slot index tables for the on-device one-hot builds,
            # prefetched once and shared by B1 and B2
            dPall = cp.tile([128, NST, T], dt.float16)
            nc.sync.dma_start(dPall[:], dPt.ap().rearrange("s p t -> p s t"))
            dFall = cp.tile([128, NST, NE], dt.float16)
            nc.sync.dma_start(dFall[:], dFt.ap().rearrange("s p e -> p s e"))

            # ---------------- phase A: h1/score table ----------------
            with (
                tc.tile_pool(name="xa", bufs=3) as xap,
                tc.tile_pool(name="ha", bufs=3) as hap,
                tc.tile_pool(name="pa", bufs=2, space="PSUM") as pap,
            ):
                for ii in range((MCH + 1) // 2):
                    i0 = ii * 2
                    nch = min(2, MCH - i0)
                    xf = xap.tile([128, 2, 256], dt.float16, tag="xf")
                    nc.sync.dma_start(
                        xf[:, :, 0:nch * 128],
                        xT.ap()[:, i0 * 128:(i0 + nch) * 128]
                        .rearrange("(j p) c -> p j c", p=128),
                    )
                    for c in range(nch):
                        i = i0 + c
                        rows = min(128, N - i * 128)
                        ph = pap.tile([128, COL_EX], dt.float32)
                        for j in range(2):
                            for s0, s1 in ((0, 512), (512, 1024),
                                           (1024, COL_EX)):
                                nc.tensor.matmul(
                                    ph[:, s0:s1],
                                    xf[:, j, c * 128:(c + 1) * 128],
                                    w1i_sb[:, j, s0:s1],
                                    start=(j == 0), stop=(j == 1))
                        h1s = hap.tile([128, ROW1], dt.float16, tag="h1s")
                        nc.vector.tensor_copy(h1s[:, 0:520], ph[:, 0:520])
                        nc.scalar.activation(h1s[:, 520:COL_EX],
                                             ph[:, 520:COL_EX], Act.Copy)
                        nc.scalar.dma_start(
                            table1.ap()[i * 128:i * 128 + rows, 0:COL_EX],
                            h1s[0:rows, 0:COL_EX]
                        )
                if stage != "full":
                    da = xap.tile([128, ROW1], dt.float16, tag="da")
                    nc.sync.dma_start(da[:], table1.ap()[0:128, :])
                    nc.sync.dma_start(dbgA[:], da[:])

            # dst-local a_d for this core's 1280 slots
            with tc.tile_pool(name="adg", bufs=1) as adgp:
                idd = adgp.tile([128, NST * 8], dt.int16)
                nc.sync.dma_start(idd[:], idxd[:])
                adg = adgp.tile([128, NST, ROW1], dt.float16)
                nc.gpsimd.dma_gather(adg[:], table1.ap(), idd[:],
                                     NST * 128, NST * 128, ROW1,
                                     single_packet=False)
                nc.vector.tensor_copy(adloc_sb[:],
                                      adg[:, :, COL_AD:COL_AD + 8])

            # ---------------- phase B1: layer-1 edge pass ----------------
            run_b1 = stage != "A"
            with (
                tc.tile_pool(name="ixp", bufs=2) as ixp,
                tc.tile_pool(name="ssp", bufs=2) as ssp,
                tc.tile_pool(name="stp", bufs=2) as stp,
                tc.tile_pool(name="gp", bufs=3) as gp,
                tc.tile_pool(name="scp", bufs=2) as scp,
                tc.tile_pool(name="adp", bufs=2, space="PSUM") as adpp,
                tc.tile_pool(name="up", bufs=1, space="PSUM") as upp,
                tc.tile_pool(name="o1p", bufs=1) as o1p,
                tc.tile_pool(name="etp", bufs=1) as etp,
                tc.tile_pool(name="tpp", bufs=2, space="PSUM") as tpp,
                tc.tile_pool(name="h2pp", bufs=1, space="PSUM") as h2pp,
                tc.tile_pool(name="tps", bufs=2) as tpsp,
            ):
                def b1_front(s):
                    """gather + one-hot build + logits + scale for supertile s"""
                    ix = ixp.tile([128, T * 8], dt.int16, tag="ix")
                    nc.sync.dma_start(ix[:], idx1.ap()[s])
                    S_sb = ssp.tile([128, T, 128], dt.float16, tag="S")
                    nc.vector.tensor_tensor(
                        S_sb[:],
                        iota_sb[:, None, :].broadcast_to([128, T, 128]),
                        dPall[:, s, :, None].broadcast_to([128, T, 128]),
                        Alu.is_equal)
                    ST_sb = stp.tile([128, NE], dt.float16, tag="ST")
                    nc.vector.tensor_tensor(
                        ST_sb[:], dFall[:, s, :],
                        pidx_sb[:].broadcast_to([128, NE]),
                        Alu.is_equal)
                    g = gp.tile([128, T, ROW1], dt.float16, tag="g")
                    nc.gpsimd.dma_gather(g[:], table1.ap(), ix[:], NE, NE,
                                         ROW1, single_packet=False)
                    # a_d expand: adps[e, (t h)] = S^T-matmul
                    adps = adpp.tile([128, T * 8], dt.float32, tag="adps")
                    STv = ST_sb[:].rearrange("p (t e) -> p t e", t=T)
                    for t in range(T):
                        nc.tensor.matmul(adps[:, t * 8:(t + 1) * 8],
                                         STv[:, t, :], adloc_sb[:, s, :],
                                         start=True, stop=True)
                    # logits -> exp
                    sc = scp.tile([128, T * 8], dt.float32, tag="sc")
                    nc.vector.tensor_copy(sc[:], g[:, :, COL_AS:COL_AS + 8])
                    nc.vector.tensor_tensor(sc[:], sc[:], adps[:], Alu.add)
                    lr = scp.tile([128, T * 8], dt.float32, tag="lr")
                    nc.vector.tensor_scalar_mul(lr[:], sc[:], NEG)
                    nc.vector.tensor_max(lr[:], lr[:], sc[:])
                    ex = scp.tile([128, T * 8], dt.float16, tag="ex")
                    nc.scalar.activation(ex[:], lr[:], Act.Exp)
                    # scale features by exp; stash exp in the ex slot
                    gv = g[:].rearrange("p t (h c) -> p t h c", c=C1)
                    exv = ex[:].rearrange("p (t h) -> p t h", h=8)
                    nc.vector.tensor_tensor(
                        gv[:, :, 0:8, :], gv[:, :, 0:8, :],
                        exv[:, :, :, None].broadcast_to([128, T, 8, C1]),
                        Alu.mult)
                    nc.vector.tensor_copy(g[:, :, COL_EX:COL_EX + 8], exv[:])
                    return s, S_sb, g

                def b1_back(st):
                    """scatter + softmax-normalize + ELU + h2 + tp write"""
                    s, S_sb, g = st
                    u = upp.tile([128, COL_EX + 8], dt.float32, tag="u")
                    for t in range(T):
                        for s0, s1 in ((0, 512), (512, 1024),
                                       (1024, COL_EX + 8)):
                            nc.tensor.matmul(u[:, s0:s1], S_sb[:, t, :],
                                             g[:, t, s0:s1],
                                             start=(t == 0), stop=(t == T - 1))
                    rc = scp.tile([128, 8], dt.float32, tag="rc")
                    nc.vector.tensor_scalar(
                        out=rc[:], in0=u[:, COL_EX:COL_EX + 8],
                        scalar1=1e-12, scalar2=None, op0=Alu.add)
                    nc.vector.reciprocal(rc[:], rc[:])
                    o1 = o1p.tile([128, D1], dt.float32, tag="o1")
                    o1v = o1[:].rearrange("p (h c) -> p h c", c=C1)
                    uv = u[:, 0:D1].rearrange("p (h c) -> p h c", c=C1)
                    nc.vector.tensor_tensor(
                        o1v[:], uv[:],
                        rc[:, :, None].broadcast_to([128, 8, C1]), Alu.mult)
                    nc.vector.tensor_add(o1[:], o1[:], b1_sb[:])
                    # ELU
                    r = o1p.tile([128, D1], dt.float32, tag="relu")
                    nc.scalar.activation(r[:], o1[:], Act.Relu)
                    nc.vector.tensor_sub(o1[:], o1[:], r[:])
                    ee = o1p.tile([128, D1], dt.float32, tag="ee")
                    nc.scalar.activation(ee[:], o1[:], Act.Exp)
                    elu = o1p.tile([128, D1], dt.float32, tag="elu")
                    nc.vector.scalar_tensor_tensor(elu[:], ee[:], -1.0, r[:],
                                                   Alu.add, Alu.add)
                    # h2 = elu @ W2e via 8 transposes
                    eluT = etp.tile([128, 8, 128], dt.float16, tag="eluT")
                    for j in range(8):
                        tp_ps = tpp.tile([128, 128], dt.float32, tag="tp")
                        nc.tensor.transpose(tp_ps[:],
                                            elu[:, j * 128:(j + 1) * 128],
                                            eye_sb[:])
                        nc.scalar.activation(eluT[:, j, :], tp_ps[:], Act.Copy)
                    h2p = h2pp.tile([128, 66], dt.float32, tag="h2p")
                    for j in range(8):
                        nc.tensor.matmul(h2p[:], eluT[:, j, :], w2e_sb[:, j, :],
                                         start=(j == 0), stop=(j == 7))
                    tp_sb = tpsp.tile([128, 65], dt.float16, tag="tpsb")
                    nc.vector.tensor_copy(tp_sb[:], h2p[:, 0:65])
                    nc.vector.tensor_copy(ad2_sb[:, s:s + 1], h2p[:, 65:66])
                    nc.scalar.dma_start(
                        tpl.ap()[s * 128:(s + 1) * 128, 0:65], tp_sb[:])
                    if stage != "full":
                        tp_dbg = tpsp.tile([128, 66], dt.float32, tag="tpdbg")
                        nc.vector.tensor_copy(tp_dbg[:], h2p[:])
                        nc.sync.dma_start(
                            dbgT.ap()[s * 128:(s + 1) * 128, :], tp_dbg[:])

                prev = None
                for s in range(NST if run_b1 else 0):
                    st = b1_front(s)
                    if prev is not None:
                        b1_back(prev)
                    prev = st
                if prev is not None:
                    b1_back(prev)

                if run_b1 and stage in ("full", "AG", "B2D"):
                    nc.gpsimd.collective_compute(
                        "AllGather", Alu.bypass,
                        ins=[tpl[:]],
                        outs=[tpg[:]],
                        replica_groups=[list(range(NCORES))],
                    )

            # ---------------- phase B2: layer-2 edge pass ----------------
            with (
                tc.tile_pool(name="ixp2", bufs=2) as ixp2,
                tc.tile_pool(name="ssp2", bufs=2) as ssp2,
                tc.tile_pool(name="stp2", bufs=2) as stp2,
                tc.tile_pool(name="g2p", bufs=3) as g2p,
                tc.tile_pool(name="sc2p", bufs=2) as sc2p,
                tc.tile_pool(name="adp2", bufs=2, space="PSUM") as adpp2,
                tc.tile_pool(name="u2p", bufs=2, space="PSUM") as u2pp,
                tc.tile_pool(name="o2p", bufs=2) as o2p,
            ):
                def b2_front(s):
                    ix2 = ixp2.tile([128, T * 8], dt.int16, tag="ix2")
                    nc.sync.dma_start(ix2[:], idx2.ap()[s])
                    S_sb = ssp2.tile([128, T, 128], dt.float16, tag="S2")
                    nc.vector.tensor_tensor(
                        S_sb[:],
                        iota_sb[:, None, :].broadcast_to([128, T, 128]),
                        dPall[:, s, :, None].broadcast_to([128, T, 128]),
                        Alu.is_equal)
                    ST_sb = stp2.tile([128, NE], dt.float16, tag="ST2")
                    nc.vector.tensor_tensor(
                        ST_sb[:], dFall[:, s, :],
                        pidx_sb[:].broadcast_to([128, NE]),
                        Alu.is_equal)
                    g2 = g2p.tile([128, T, ROW2], dt.float16, tag="g2")
                    nc.gpsimd.dma_gather(g2[:], tpg.ap(), ix2[:], NE, NE,
                                         ROW2, single_packet=False)
                    ad2e = adpp2.tile([128, T], dt.float32, tag="ad2e")
                    STv = ST_sb[:].rearrange("p (t e) -> p t e", t=T)
                    for t in range(T):
                        nc.tensor.matmul(ad2e[:, t:t + 1], STv[:, t, :],
                                         ad2_sb[:, s:s + 1],
                                         start=True, stop=True)
                    sc2 = sc2p.tile([128, T], dt.float32, tag="sc2")
                    nc.vector.tensor_copy(sc2[:], g2[:, :, 64])
                    nc.vector.tensor_tensor(sc2[:], sc2[:], ad2e[:], Alu.add)
                    l2 = sc2p.tile([128, T], dt.float32, tag="l2")
                    nc.vector.tensor_scalar_mul(l2[:], sc2[:], NEG)
                    nc.vector.tensor_max(l2[:], l2[:], sc2[:])
                    e2 = sc2p.tile([128, T], dt.float16, tag="e2")
                    nc.scalar.activation(e2[:], l2[:], Act.Exp)
                    nc.vector.tensor_tensor(
                        g2[:, :, 0:C2], g2[:, :, 0:C2],
                        e2[:, :, None].broadcast_to([128, T, C2]), Alu.mult)
                    nc.vector.tensor_copy(g2[:, :, C2 + 1], e2[:])
                    return s, S_sb, g2, ad2e, sc2, l2, e2

                def b2_back(st):
                    s, S_sb, g2, ad2e, sc2, l2, e2 = st
                    # one accumulation group per t (PSUM groups are
                    # bank-granular): cols 0:64 feats, 64 junk, 65 denom
                    u2 = u2pp.tile([128, C2 + 2], dt.float32, tag="u2")
                    for t in range(T):
                        nc.tensor.matmul(u2[:], S_sb[:, t, :],
                                         g2[:, t, 0:C2 + 2],
                                         start=(t == 0), stop=(t == T - 1))
                    if stage == "B2D" and s == 0:
                        nc.sync.dma_start(
                            dbgG2[:], g2[:].rearrange("p t c -> p (t c)"))
                        sdump = sc2p.tile([128, 4 * T], dt.float32, tag="sd")
                        nc.vector.tensor_copy(sdump[:, 0:T], ad2e[:])
                        nc.vector.tensor_copy(sdump[:, T:2 * T], sc2[:])
                        nc.vector.tensor_copy(sdump[:, 2 * T:3 * T], l2[:])
                        nc.vector.tensor_copy(sdump[:, 3 * T:4 * T], e2[:])
                        nc.sync.dma_start(dbgS2[:], sdump[:])
                        u2dump = o2p.tile([128, C2 + 2], dt.float32, tag="ud")
                        nc.vector.tensor_copy(u2dump[:], u2[:])
                        nc.sync.dma_start(dbgU2[:], u2dump[:])
                    rc2 = sc2p.tile([128, 1], dt.float32, tag="rc2")
                    nc.vector.tensor_scalar(
                        out=rc2[:], in0=u2[:, C2 + 1:C2 + 2],
                        scalar1=1e-12, scalar2=None, op0=Alu.add)
                    nc.vector.reciprocal(rc2[:], rc2[:])
                    o2 = o2p.tile([128, C2], dt.float32, tag="o2")
                    nc.vector.tensor_scalar_mul(o2[:], u2[:, 0:C2],
                                                rc2[:, 0:1])
                    nc.vector.tensor_add(o2[:], o2[:], b2_sb[:])
                    nc.sync.dma_start(
                        out.ap()[s * 128:(s + 1) * 128, :], o2[:])

                prev2 = None
                for s in range(NST if stage in ("full", "B2D") else 0):
                    st2 = b2_front(s)
                    if prev2 is not None:
                        b2_back(prev2)
                    prev2 = st2
                if prev2 is not None:
                    b2_back(prev2)

    nc.compile()
    legalize_waits(nc)
    return nc


def _get_prog(T):
    import os
    stage = os.environ.get("KERNEL_STAGE", "full")
    key = (T, stage)
    if key not in _prog_cache:
        _prog_cache[key] = _build(T, stage)
    return _prog_cache[key]


# ------------------------------------------------------------------ kernel
def kernel(x, edge_index, W1, att_src1, att_dst1, b1, W2, att_src2, att_dst2,
           b2, _run_kwargs=None):
    edge_index = np.asarray(edge_index)
    es = _edge_struct(edge_index)
    params = _host_params(x, W1, att_src1, att_dst1, b1, W2, att_src2,
                          att_dst2, b2)
    T = es["T"]
    nc = _get_prog(T)

    in_maps = []
    for k in range(NCORES):
        m = dict(params)
        m["idx1"] = es["idx1"][k]
        m["idx2"] = es["idx2"][k]
        m["idxd"] = es["idxd"][k]
        m["dP"] = es["dP"][k]
        m["dF"] = es["dF"][k]
        in_maps.append(m)

    res = run_bass_kernel_spmd(nc, in_maps, list(range(NCORES)),
                               **(_run_kwargs or {}))
    full = np.zeros((N, C2), f32)
    for k in range(NCORES):
        ok = es["valid"][k]
        full[es["node_at"][k][ok]] = res.results[k]["out"][ok]
    kernel.last_results = res
    return full
